# revision 52
# baseline (speedup 1.0000x reference)
"""Trainium2 Bass kernel for nn_ClassifierGCN (GCN conv -> z@z^T -> MLP -> sigmoid).

Contract: kernel(**inputs) takes the FULL unsharded inputs (numpy), distributes
across 8 NeuronCores internally, and returns the FULL output (numpy, f32).

Strategy (8 cores):
  - Host: build the dense edge-COUNT matrix C[src, dst] (pure index
    preprocessing; small integers -> exact in fp8, halving the adjacency
    stream), column-shard it 640 dst nodes per core (8 whole graphs/core).
    The D^-1/2 normalization ships as two tiny f32 dinv tensors.
  - Device, phase A (per core): h' = dinv_src * (x @ Wg) for ALL nodes,
    aggT_raw = h'.T @ C_slice (bf16 x fp8 matmul, f32 accum),
    zT = relu(dinv_dst * aggT_raw + bg)   [128 latent x 640 local nodes]
  - Device, phase B: per local graph g, G = z_g @ z_g^T (symmetric, [80,80]),
    flatten to DRAM; chunked AllGather -> all 64 graphs' G on every core.
  - Device, phase C: fc1/fc2 tensor-sharded along the 2n^2=12800 hidden dim
    (1600 per core): y1 = relu(Gall @ W1[:, s] + b1[s]);
    partial = y1 @ W2[s, :]  (+ b2 folded into core 0's partial, zeros elsewhere);
    chunked ReduceScatter(sum) scatters graphs back to their home cores ->
    sigmoid -> each core outputs its own 8 graphs; host concatenates.
  - bf16 for all large operands (f32 PSUM accumulation); weight streams are
    coarsened (2 K-tiles per DMA) and alternated across the two HWDGE rings,
    with deep SBUF prefetch so the streams run during phase A / the AllGather.
"""

import numpy as np
import ml_dtypes

import bass_rust
import concourse.bass as bass
import concourse.mybir as mybir
import concourse.tile as tile
from concourse.bass_utils import run_bass_kernel_spmd
from concourse.masks import make_identity
from concourse.tile_rust import add_dep_helper

# Problem shapes (hardcoded per contract).
N_NEURONS = 80
TBL = 256
LATENT = 128
N_GRAPHS = 64
N_NODES = 5120
N_CORES = 8
GPC = N_GRAPHS // N_CORES          # graphs per core = 8
DPC = N_NODES // N_CORES           # dst nodes per core = 640
N2 = N_NEURONS * N_NEURONS         # 6400
HID = 2 * N2                       # 12800
HS = HID // N_CORES                # hidden slice per core = 1600

DT = mybir.dt.bfloat16             # compute dtype for large operands
NP_DT = ml_dtypes.bfloat16
F32 = mybir.dt.float32

K_TILES_NODES = N_NODES // 128     # 40
K_TILES_N2 = N2 // 128             # 50
# fc2 contraction: 1600 = 12*128 + 64
K_TILES_HS = [(k * 128, 128) for k in range(12)] + [(1536, 64)]
N_CHUNKS_1600 = [(0, 512), (512, 512), (1024, 512), (1536, 64)]


def _fix_excess_waits(nc):
    """This container's walrus rejects >1 sem-wait on CTRL-class instructions.
    Tile's end-of-context Drain can carry several; move the excess onto NoOp
    carriers inserted just before, same engine, program order preserved."""
    n_fix = 0
    for f in nc.m.functions:
        for bb in f.blocks:
            out, changed = [], False
            for inst in bb.instructions:
                si = inst.sync_info
                waits = list(si.on_wait) if si is not None and si.on_wait else []
                if len(waits) > 1:
                    for w in waits[:-1]:
                        nop = mybir.InstNoOp(name=f"I-waitfix-{n_fix}", ins=[], outs=[])
                        n_fix += 1
                        nop.engine = inst.engine
                        nop.sync_info = bass_rust.SyncInfo(on_wait=[w], on_update=[])
                        out.append(nop)
                    si.on_wait = waits[-1:]
                    changed = True
                out.append(inst)
            if changed:
                bb.instructions = out
    return n_fix


def build_nc():
    nc = bass.Bass(num_devices=N_CORES)

    xT = nc.dram_tensor("xT", [TBL, N_NODES], DT, kind="ExternalInput")
    wg = nc.dram_tensor("wg", [TBL, LATENT], DT, kind="ExternalInput")
    bg = nc.dram_tensor("bg", [LATENT, 1], F32, kind="ExternalInput")
    ats = nc.dram_tensor("ats", [N_NODES, DPC], mybir.dt.float8e4,
                         kind="ExternalInput")
    dinv_s = nc.dram_tensor("dinv_s", [128, K_TILES_NODES], F32,
                            kind="ExternalInput")
    dinv_d = nc.dram_tensor("dinv_d", [128, DPC], F32, kind="ExternalInput")
    w1s = nc.dram_tensor("w1s", [N2, HS], DT, kind="ExternalInput")
    b1s = nc.dram_tensor("b1s", [1, HS], DT, kind="ExternalInput")
    w2s = nc.dram_tensor("w2s", [HS, N2], DT, kind="ExternalInput")
    b2s = nc.dram_tensor("b2s", [1, N2], DT, kind="ExternalInput")
    y = nc.dram_tensor("y", [GPC, N2], F32, kind="ExternalOutput")

    RG = [list(range(N_CORES))]

    with tile.TileContext(nc) as tc:
        with (
            # Weight-stream pools first so their SBUF ranges never overlap the
            # phase-A pools -> prefetch can run from t=0.
            tc.tile_pool(name="w1p", bufs=14) as w1p,
            tc.tile_pool(name="w2p", bufs=6) as w2p,
            tc.tile_pool(name="const", bufs=1) as constp,
            tc.tile_pool(name="persist", bufs=1) as persist,
            tc.tile_pool(name="dram", bufs=1, space="DRAM") as dramp,
        ):
            # Constants.
            ident = constp.tile([64, 64], DT)
            make_identity(nc, ident[:])
            ones1 = constp.tile([1, 64], DT)
            nc.gpsimd.memset(ones1[:], 1.0)
            bg_sb = constp.tile([LATENT, 1], F32)
            dinv_s_sb = constp.tile([128, K_TILES_NODES], F32)
            dinv_d_sb = constp.tile([128, DPC], F32)
            b1_sb = constp.tile([1, HS], DT)
            b2_sb = constp.tile([1, N2], DT)

            # Persistent SBUF tensors.
            zT = persist.tile([128, DPC], DT)                        # [latent, local node]
            gT_big = persist.tile([128, K_TILES_N2 * 64], DT)        # vec(G) K-tiles x 64 graphs
            y1T_big = persist.tile([128, len(K_TILES_HS) * 64], DT)  # y1^T K-tiles x 64 graphs
            y1_sb = persist.tile([N_GRAPHS, HS], DT)

            # DRAM bounce buffers for the (chunked) collectives.
            AG_CHUNKS = 2
            AG_W = [1920, 4480]
            AG_R = [24, 56]
            AG_R0 = [0, 24]
            g_loc = [dramp.tile([GPC, w], DT, name=f"g_loc{c}")
                     for c, w in enumerate(AG_W)]
            g_all = [dramp.tile([N_GRAPHS, w], DT, addr_space="Shared",
                                name=f"g_all{c}")
                     for c, w in enumerate(AG_W)]
            RS_W = [3200, 3200]         # RS after fc2 chunks 1 and 3
            RS_OF = [0, 3200]
            y_loc = [dramp.tile([N_GRAPHS, w], DT, name=f"y_loc{c}")
                     for c, w in enumerate(RS_W)]
            y_red = [dramp.tile([GPC, w], DT, name=f"y_red{c}")
                     for c, w in enumerate(RS_W)]

            # ---- Phase A1: h = x @ Wg  (all 5120 nodes) ----
            with (
                tc.tile_pool(name="xt", bufs=1) as xtp,
                tc.tile_pool(name="hps", bufs=2, space="PSUM") as hps,
            ):
                h_big = xtp.tile([128, K_TILES_NODES * 128], DT, tag="hbig")
                xt_sb = xtp.tile([128, 2, N_NODES], DT, tag="xt")
                wg_sb = xtp.tile([128, 2, LATENT], DT, tag="wg")
                nc.sync.dma_start(wg_sb[:, 0, :], wg[0:128, :])
                nc.sync.dma_start(wg_sb[:, 1, :], wg[128:256, :])
                for cc in range(4):
                    cs, ce = cc * 1280, (cc + 1) * 1280
                    nc.sync.dma_start(xt_sb[:, 0, cs:ce], xT[0:128, cs:ce])
                    nc.sync.dma_start(xt_sb[:, 1, cs:ce], xT[128:256, cs:ce])

                for m in range(K_TILES_NODES):
                    ph = hps.tile([128, 128], F32)
                    nc.tensor.matmul(
                        ph[:],
                        lhsT=xt_sb[:, 0, m * 128:(m + 1) * 128],
                        rhs=wg_sb[:, 0, :],
                        start=True, stop=False,
                    )
                    nc.tensor.matmul(
                        ph[:],
                        lhsT=xt_sb[:, 1, m * 128:(m + 1) * 128],
                        rhs=wg_sb[:, 1, :],
                        start=False, stop=True,
                    )
                    nc.vector.tensor_copy(h_big[:, m * 128:(m + 1) * 128], ph[:])

                # ---- Phase A2: aggT = h.T @ ATs  -> zT = relu(aggT + bg) ----
                with (
                    tc.tile_pool(name="atp", bufs=2) as atp,
                    tc.tile_pool(name="aggps", bufs=1, space="PSUM") as aggps,
                ):
                    agg = aggps.tile([128, DPC], F32)
                    for kb in range(8):
                        at5 = atp.tile([128, 5, DPC], DT)
                        src = ats[kb * 640:(kb + 1) * 640, :].rearrange(
                            "(a b) c -> b a c", a=5)
                        nc.sync.dma_start(at5[:], src)
                        for j in range(5):
                            k = kb * 5 + j
                            st = (k == 0)
                            sp = (k == K_TILES_NODES - 1)
                            lhs = h_big[:, k * 128:(k + 1) * 128]
                            nc.tensor.matmul(agg[:, 0:512], lhsT=lhs,
                                             rhs=at5[:, j, 0:512],
                                             start=st, stop=sp)
                            nc.tensor.matmul(agg[:, 512:640], lhsT=lhs,
                                             rhs=at5[:, j, 512:640],
                                             start=st, stop=sp)
                    nc.scalar.activation(zT[:], agg[:],
                                         mybir.ActivationFunctionType.Relu,
                                         bias=bg_sb[:, 0:1])

            # ---- Phase B: per-graph G = z z^T, flatten, chunked AllGather ----
            with (
                tc.tile_pool(name="gps", bufs=2, space="PSUM") as gps,
                tc.tile_pool(name="gsb", bufs=1) as gsbp,
            ):
                gsb_all = gsbp.tile([N_NEURONS, GPC * N_NEURONS], DT)
                for g in range(GPC):
                    gp = gps.tile([N_NEURONS, N_NEURONS], F32)
                    zg = zT[:, g * N_NEURONS:(g + 1) * N_NEURONS]
                    nc.tensor.matmul(gp[:], lhsT=zg, rhs=zg, start=True, stop=True)
                    nc.vector.tensor_copy(
                        gsb_all[:, g * N_NEURONS:(g + 1) * N_NEURONS], gp[:])
                for c in range(AG_CHUNKS):
                    # g_loc[c][g, r*80+col] = G_g[AG_R0[c] + r, col]
                    dst = g_loc[c][:, :].rearrange("g (r c) -> r g c", r=AG_R[c])
                    nc.sync.dma_start(
                        dst,
                        gsb_all[AG_R0[c]:AG_R0[c] + AG_R[c], :].rearrange(
                            "r (g c) -> r g c", g=GPC))
                    nc.gpsimd.collective_compute(
                        "AllGather", mybir.AluOpType.bypass, replica_groups=RG,
                        ins=[g_loc[c].opt()], outs=[g_all[c].opt()],
                    )

            # ---- Phase C0: transpose Gall into [128 x 64] K-tiles ----
            with (
                tc.tile_pool(name="gallp", bufs=4) as gallp,
                tc.tile_pool(name="tps", bufs=2, space="PSUM") as tps,
            ):
                for blk in range(10):           # 10 loads of [64, 640]
                    c = 0 if blk < 3 else 1
                    b = blk if blk < 3 else blk - 3
                    ga = gallp.tile([N_GRAPHS, 640], DT)
                    nc.sync.dma_start(ga[:], g_all[c][:, b * 640:(b + 1) * 640])
                    for j in range(5):
                        t = blk * 5 + j
                        tp = tps.tile([128, N_GRAPHS], DT)
                        nc.tensor.transpose(tp[:], ga[:, j * 128:(j + 1) * 128],
                                            ident[:])
                        nc.vector.tensor_copy(gT_big[:, t * 64:(t + 1) * 64], tp[:])

                # ---- Phase C1: y1 = relu(Gall @ W1s + b1s) ----
                with tc.tile_pool(name="y1ps", bufs=1, space="PSUM") as y1psp:
                    y1ps = y1psp.tile([N_GRAPHS, HS], F32)
                    for (n0, nw) in N_CHUNKS_1600:
                        nc.tensor.matmul(y1ps[:, n0:n0 + nw], lhsT=ones1[:],
                                         rhs=b1_sb[:, n0:n0 + nw],
                                         start=True, stop=False)
                    for kp in range(K_TILES_N2 // 2):
                        w1t = wpool.tile([128, 2, HS], DT, tag="w")
                        eng = nc.scalar if kp % 2 == 0 else nc.sync
                        w1_dma = eng.dma_start(
                            w1t[:],
                            w1s[kp * 256:(kp + 1) * 256, :].rearrange(
                                "(a b) c -> b a c", a=2))
                        if kp <= 1:
                            # keep the first weight-pair transfers from landing
                            # ahead of the phase-A-critical first xT chunk
                            add_dep_helper(w1_dma.ins, nc._first_xt_dma,
                                           sync=True,
                                           reason="w-stream after first xT")
                        for kk in range(2):
                            k = kp * 2 + kk
                            lhs = gT_big[:, k * 64:(k + 1) * 64]
                            for (n0, nw) in N_CHUNKS_1600:
                                nc.tensor.matmul(y1ps[:, n0:n0 + nw], lhsT=lhs,
                                                 rhs=w1t[:, kk, n0:n0 + nw],
                                                 start=False,
                                                 stop=(k == K_TILES_N2 - 1))
                    for (n0, nw) in N_CHUNKS_1600:
                        nc.scalar.activation(y1_sb[:, n0:n0 + nw],
                                             y1ps[:, n0:n0 + nw],
                                             mybir.ActivationFunctionType.Relu)

                # ---- Phase C2: transpose y1 into K-tiles ----
                for t, (k0, kw) in enumerate(K_TILES_HS):
                    tp = tps.tile([128, N_GRAPHS], DT)
                    nc.tensor.transpose(tp[0:kw, :], y1_sb[:, k0:k0 + kw], ident[:])
                    nc.vector.tensor_copy(y1T_big[0:kw, t * 64:(t + 1) * 64], tp[0:kw, :])

            # ---- Phase C3: fc2 partial = y1 @ W2s (+ b2 on core 0), chunked;
            #      ReduceScatter after chunks 1 and 3, sigmoid + store ----
            with (
                tc.tile_pool(name="p2ps", bufs=2, space="PSUM") as p2psp,
                tc.tile_pool(name="y2sb", bufs=2) as y2sbp,
                tc.tile_pool(name="sig", bufs=4) as sigp,
            ):
                for c in range(4):
                    c0 = c * 1600
                    p2 = p2psp.tile([N_GRAPHS, 1600], F32)
                    for (n0, nw) in N_CHUNKS_1600:
                        nc.tensor.matmul(p2[:, n0:n0 + nw], lhsT=ones1[:],
                                         rhs=b2_sb[:, c0 + n0:c0 + n0 + nw],
                                         start=True, stop=False)
                    for tp2 in range(7):
                        eng = nc.scalar if (c * 7 + tp2) % 2 == 0 else nc.sync
                        if tp2 < 6:
                            w2t = wpool.tile([128, 2, 1600], DT, tag="w")
                            eng.dma_start(
                                w2t[:],
                                w2s[tp2 * 256:(tp2 + 1) * 256,
                                    c0:c0 + 1600].rearrange("(a b) c -> b a c", a=2))
                            pieces = [(tp2 * 2, 0, 128), (tp2 * 2 + 1, 1, 128)]
                        else:
                            w2t = wpool.tile([128, 2, 1600], DT, tag="w")
                            eng.dma_start(w2t[0:64, 0, :],
                                          w2s[1536:1600, c0:c0 + 1600])
                            pieces = [(12, 0, 64)]
                        for (t, kk, kw) in pieces:
                            lhs = y1T_big[0:kw, t * 64:(t + 1) * 64]
                            for (n0, nw) in N_CHUNKS_1600:
                                nc.tensor.matmul(p2[:, n0:n0 + nw], lhsT=lhs,
                                                 rhs=w2t[0:kw, kk, n0:n0 + nw],
                                                 start=False, stop=(t == 12))
                    y2sb = y2sbp.tile([N_GRAPHS, 1600], DT)
                    for (n0, nw) in N_CHUNKS_1600:
                        nc.vector.tensor_copy(y2sb[:, n0:n0 + nw],
                                              p2[:, n0:n0 + nw])
                    r = c // 2
                    rc0 = c0 - RS_OF[r]
                    nc.sync.dma_start(y_loc[r][:, rc0:rc0 + 1600], y2sb[:])
                    if c % 2 == 1:
                        w = RS_W[r]
                        nc.gpsimd.collective_compute(
                            "ReduceScatter", mybir.AluOpType.add, replica_groups=RG,
                            ins=[y_loc[r].opt()], outs=[y_red[r].opt()],
                        )
                        # sigmoid over all 128 partitions: [8, w] -> [128, w/16]
                        w16 = w // 16
                        ys = sigp.tile([128, 200], DT, tag="ys")
                        nc.sync.dma_start(
                            ys[:, 0:w16],
                            y_red[r][:, :].rearrange("g (j t) -> g j t", j=16))
                        yo = sigp.tile([128, 200], F32, tag="yo")
                        nc.scalar.activation(yo[:, 0:w16], ys[:, 0:w16],
                                             mybir.ActivationFunctionType.Sigmoid)
                        nc.sync.dma_start(
                            y[:, RS_OF[r]:RS_OF[r] + w].rearrange(
                                "g (j t) -> g j t", j=16), yo[:, 0:w16])

    _fix_excess_waits(nc)
    return nc


_NC_CACHE = None


def _get_nc():
    global _NC_CACHE
    if _NC_CACHE is None:
        _NC_CACHE = build_nc()
    return _NC_CACHE


def prep_in_maps(x, edge_index, Wg, bg, W1, b1, W2, b2):
    x = np.asarray(x, np.float32)
    edge_index = np.asarray(edge_index)
    Wg = np.asarray(Wg, np.float32)
    bg = np.asarray(bg, np.float32)
    W1 = np.asarray(W1, np.float32)
    b1 = np.asarray(b1, np.float32)
    W2 = np.asarray(W2, np.float32)
    b2 = np.asarray(b2, np.float32)

    src = edge_index[0].astype(np.int64)
    dst = edge_index[1].astype(np.int64)

    # Degree / normalization (index preprocessing, matches reference formula).
    deg = np.bincount(dst, minlength=N_NODES).astype(np.float32)
    dinv = np.where(deg > 0, 1.0 / np.sqrt(np.maximum(deg, 1.0)), 0.0).astype(np.float32)

    # Dense edge-count matrix, laid out [src, dst]. Counts are small integers
    # -> exactly representable in fp8; the dinv normalization is applied on
    # device in f32, so this halves the adjacency stream with no extra error.
    # (bincount over flat indices is ~10x faster than np.add.at here)
    counts = np.bincount(src * N_NODES + dst, minlength=N_NODES * N_NODES)
    at = counts.astype(ml_dtypes.float8_e4m3).reshape(N_NODES, N_NODES)
    dinv_s_np = np.ascontiguousarray(
        dinv.reshape(K_TILES_NODES, 128).T)          # [128, 40]

    xT = np.ascontiguousarray(x.T).astype(NP_DT)
    wg_np = Wg.astype(NP_DT)
    bg_np = np.ascontiguousarray(bg.reshape(LATENT, 1))

    in_maps = []
    for c in range(N_CORES):
        s0 = c * HS
        b2c = b2 if c == 0 else np.zeros_like(b2)
        in_maps.append({
            "xT": xT,
            "wg": wg_np,
            "bg": bg_np,
            "ats": np.ascontiguousarray(at[:, c * DPC:(c + 1) * DPC]),
            "dinv_s": dinv_s_np,
            "dinv_d": np.ascontiguousarray(np.broadcast_to(
                dinv[c * DPC:(c + 1) * DPC], (128, DPC))),
            "w1s": np.ascontiguousarray(W1[:, s0:s0 + HS]).astype(NP_DT),
            "b1s": np.ascontiguousarray(b1[s0:s0 + HS].reshape(1, HS)).astype(NP_DT),
            "w2s": np.ascontiguousarray(W2[s0:s0 + HS, :]).astype(NP_DT),
            "b2s": np.ascontiguousarray(b2c.reshape(1, N2)).astype(NP_DT),
        })
    return in_maps


def kernel(x, edge_index, Wg, bg, W1, b1, W2, b2):
    in_maps = prep_in_maps(x, edge_index, Wg, bg, W1, b1, W2, b2)
    nc = _get_nc()
    res = run_bass_kernel_spmd(nc, in_maps, core_ids=list(range(N_CORES)))
    out = np.concatenate([res.results[c]["y"] for c in range(N_CORES)], axis=0)
    return out.reshape(-1).astype(np.float32)
